# revision 4
# baseline (speedup 1.0000x reference)
"""Trainium2 Bass kernel for nn_Convolution_77111842832763.

3D conv 5x5x5 SAME, 64->64 channels, input [2,40,40,40,64] fp32, plus an
irrep-wise linear self-connection (folded into the conv's center tap).

Strategy (8 NeuronCores, data-parallel):
  - Shard: core = batch(2) x x-chunk(4); each core computes a [10,40,40,64]
    output slab from a zero-padded [14,44,44,64] input slab (halo 2).
  - Host builds the 5^3 x 64 x 64 tensor-product kernel exactly (float64),
    folds the self-connection into the center tap, and packs per-tap weight
    blocks; weights/slabs are cast to fp16 on host (device matmuls run fp16
    with fp32 PSUM accumulation; measured end-to-end rel-err ~2.9e-4).
  - Device: TWO channel-major slab copies in SBUF [128, 14*44*44]:
      slab_z: partitions 0-63 = slab, 64-127 = slab shifted +1 z-voxel
      slab_y: partitions 0-63 = slab, 64-127 = slab shifted +1 y-row
    K=128 packing: the 125 taps become 65 matmul units per output tile:
      50 z-pair units  (dx,dy, dz in {(0,1),(2,3)})        -> slab_z
      15 y-pair units  (dx, dz=4 slice, dy in {(0,1),(2,3),(4,zero)}) -> slab_y
  - Units alternate between PE column groups 0-63/64-127 (2x column
    tiling) accumulating into psum[0:64]/psum[64:128]; the two partial
    sums are DMA'd out separately and added on host.
  - Output tile = one x-plane quarter: 10 y-rows x 40 z = 400 voxels
    (moving free dim 400, one PSUM bank).  40 tiles per core.
  - DMA: input planes on the SP HWDGE ring, weights/outputs on the ACT
    HWDGE ring (parallel rings).  A JSON post-pass splits multi-wait
    instructions (this walrus build allows one sync wait per instruction).
"""

import functools
import json
import math
from contextlib import ExitStack

import numpy as np

import concourse.bass as bass
import concourse.mybir as mybir
import concourse.tile as tile
from concourse.bass_utils import run_bass_kernel_spmd

MUL = 16
DIM = 64
NB = 8
PX, PY, PZ = 14, 44, 44          # padded slab dims
PLANE = PY * PZ                   # 1936
SLAB = PX * PLANE                 # 27104
SLAB_PAD = SLAB + 48              # tail zeros so the +1z and +44y views stay in-bounds
# unit list: ("z", dx, dy, zg) -> taps (dx,dy,2*zg)+(dx,dy,2*zg+1) via slab_z
#            ("y", dx, yg)     -> taps (dx,2*yg,4)+(dx,2*yg+1,4)   via slab_y
UNITS = [("z", dx, dy, zg) for dx in range(5) for dy in range(5) for zg in range(2)]
UNITS += [("y", dx, yg, 0) for dx in range(5) for yg in range(3)]
NU = len(UNITS)                   # 65


def _split_sync_waits_json(raw: bytes) -> bytes:
    """Hoist all but the last sync wait of each instruction onto preceding
    same-engine EventSemaphore instructions (engines execute in order, so
    this is semantically identical)."""
    m = json.loads(raw)
    ctr = 0
    for fn in m.get("functions", []):
        for blk in fn.get("blocks", []):
            out = []
            for inst in blk.get("instructions", []):
                si = inst.get("sync_info")
                ow = (si or {}).get("on_wait") or []
                if len(ow) > 1:
                    for w in ow[:-1]:
                        ctr += 1
                        out.append({
                            "debug": inst.get("debug", 0),
                            "engine": inst["engine"],
                            "ins": [],
                            "outs": [],
                            "name": f"SWX-{ctr}",
                            "opcode": "EventSemaphore",
                            "sync_info": {"on_update": [], "on_wait": [w]},
                        })
                    si["on_wait"] = [ow[-1]]
                out.append(inst)
            blk["instructions"] = out
    return json.dumps(m).encode()


def _build_tp_kernel(linear_weight: np.ndarray, weight: np.ndarray) -> np.ndarray:
    """Mirror reference.py's CG kernel construction in float64.
    Returns k[5,5,5,64,64] ([dx,dy,dz,in,out]) with the self-connection
    folded into the center tap."""
    lw = linear_weight.astype(np.float64)
    w8 = weight.astype(np.float64)
    ax = np.arange(-2.0, 3.0)
    gx, gy, gz = np.meshgrid(ax, ax, ax, indexing="ij")
    lattice = np.stack([gx, gy, gz], axis=-1)            # [5,5,5,3]
    rad = np.linalg.norm(lattice, axis=-1)
    values = np.linspace(0.0, 2.5, NB + 2)[1:-1]
    step = 2.5 / (NB + 1)
    diff = (rad[..., None] - values) / step
    den = np.maximum(1.0 - diff * diff, 1e-9)
    emb = np.where(np.abs(diff) < 1.0, 1.14136 * np.exp(2.0 - 1.0 / den), 0.0)
    n = rad[..., None]
    unit = np.where(n > 0, lattice / np.where(n > 0, n, 1.0), 0.0)
    sh = np.concatenate([np.ones((5, 5, 5, 1)), math.sqrt(3.0) * unit], -1)
    L = 125
    w = (emb.reshape(L, NB) @ w8) / float(L)             # [125, 1024]
    W = w.reshape(L, 4, MUL, MUL)
    shf = sh.reshape(L, 4)
    y0, y1 = shf[:, 0], shf[:, 1:4]
    a = 1.0 / math.sqrt(2.0 * MUL)
    eye3 = np.eye(3)
    Rss = a * W[:, 0] * y0[:, None, None]
    Rsv = a * np.einsum("luw,lm->luwm", W[:, 1], y1).reshape(L, MUL, 3 * MUL)
    Rvv = a * np.einsum("luw,l,mn->lumwn", W[:, 2], y0, eye3).reshape(L, 3 * MUL, 3 * MUL)
    Rvs = (a / math.sqrt(3.0)) * np.einsum("luw,lm->lumw", W[:, 3], y1).reshape(L, 3 * MUL, MUL)
    k = np.concatenate(
        [np.concatenate([Rss, Rsv], -1), np.concatenate([Rvs, Rvv], -1)], 1
    ).reshape(5, 5, 5, DIM, DIM)
    # self-connection: irrep-wise linear, folded into center tap
    Wl = lw.reshape(2, MUL, MUL) / math.sqrt(MUL)
    sc = np.zeros((DIM, DIM))
    sc[:MUL, :MUL] = Wl[0]
    for m in range(3):
        idx = MUL + np.arange(MUL) * 3 + m
        sc[np.ix_(idx, idx)] = Wl[1]
    k = k.copy()
    k[2, 2, 2] += sc
    return k


def _pack_weights(k: np.ndarray) -> np.ndarray:
    """[128, 65*64] fp16 per-unit weight blocks (rows 64-127 = paired tap,
    zeros when unpaired)."""
    Wp = np.zeros((128, NU * DIM), np.float64)
    for ui, u in enumerate(UNITS):
        s = ui * DIM
        if u[0] == "z":
            _, dx, dy, zg = u
            Wp[0:64, s:s + DIM] = k[dx, dy, 2 * zg]
            Wp[64:128, s:s + DIM] = k[dx, dy, 2 * zg + 1]
        else:
            _, dx, yg, _ = u
            Wp[0:64, s:s + DIM] = k[dx, 2 * yg, 4]
            if 2 * yg + 1 < 5:
                Wp[64:128, s:s + DIM] = k[dx, 2 * yg + 1, 4]
    return Wp.astype(np.float16)


def _build_slab(xb: np.ndarray, cx: int) -> np.ndarray:
    """Channel-major zero-padded fp16 slab [64, SLAB_PAD] for x-chunk cx of
    batch-slice xb [40,40,40,64]."""
    pad = np.zeros((PX, PY, PZ, DIM), np.float32)
    x0 = cx * 10 - 2
    lo, hi = max(0, x0), min(40, x0 + PX)
    pad[lo - x0:hi - x0, 2:42, 2:42, :] = xb[lo:hi]
    xs = np.ascontiguousarray(pad.transpose(3, 0, 1, 2)).reshape(DIM, SLAB)
    out = np.zeros((DIM, SLAB_PAD), np.float16)
    out[:, :SLAB] = xs.astype(np.float16)
    return out


def _build_program():
    nc = bass.Bass("TRN2", target_bir_lowering=False, debug=False)
    xs_d = nc.dram_tensor("xs", [DIM, SLAB_PAD], mybir.dt.float16, kind="ExternalInput")
    wt_d = nc.dram_tensor("wt", [128, NU * DIM], mybir.dt.float16, kind="ExternalInput")
    y_d = nc.dram_tensor("y", [128, 16000], mybir.dt.float32, kind="ExternalOutput")

    with tile.TileContext(nc) as tc:
        with ExitStack() as ctx:
            wpool = ctx.enter_context(tc.tile_pool(name="wts", bufs=1))
            ppool = ctx.enter_context(tc.tile_pool(name="planes", bufs=1))
            spool = ctx.enter_context(tc.tile_pool(name="stage", bufs=4))
            qpool = ctx.enter_context(tc.tile_pool(name="psum", bufs=2, space="PSUM"))

            wt_sb = wpool.tile([128, NU * DIM], mybir.dt.float16)
            nc.scalar.dma_start(wt_sb[:], wt_d.ap())

            # plane groups: 0-4 (gates the first output plane) and 5-13,
            # one batched DMA per slab-copy half per group
            G0, G1 = 5, PX - 5
            gz0 = ppool.tile([128, G0 * PLANE], mybir.dt.float16, name="gz0", tag="gz0")
            gz1 = ppool.tile([128, G1 * PLANE], mybir.dt.float16, name="gz1", tag="gz1")
            gy0 = ppool.tile([128, G0 * PLANE], mybir.dt.float16, name="gy0", tag="gy0")
            gy1 = ppool.tile([128, G1 * PLANE], mybir.dt.float16, name="gy1", tag="gy1")
            s0, s1 = G0 * PLANE, SLAB
            nc.sync.dma_start(gz0[0:64, :], xs_d.ap()[:, 0:s0])
            nc.sync.dma_start(gz0[64:128, :], xs_d.ap()[:, 1:s0 + 1])
            nc.sync.dma_start(gz1[0:64, :], xs_d.ap()[:, s0:s1])
            nc.sync.dma_start(gz1[64:128, :], xs_d.ap()[:, s0 + 1:s1 + 1])
            nc.scalar.dma_start(gy0[0:64, :], xs_d.ap()[:, 0:s0])
            nc.scalar.dma_start(gy0[64:128, :], xs_d.ap()[:, PY:s0 + PY])
            nc.scalar.dma_start(gy1[0:64, :], xs_d.ap()[:, s0:s1])
            nc.scalar.dma_start(gy1[64:128, :], xs_d.ap()[:, s0 + PY:s1 + PY])

            def plane(kind, i):
                g = (gz0 if i < G0 else gz1) if kind == "z" else (gy0 if i < G0 else gy1)
                off = i if i < G0 else i - G0
                return g[:].rearrange("p (x y z) -> p x y z", y=PY, z=PZ)[:, off]

            for px in range(10):
                stage = spool.tile([128, 1600], mybir.dt.float32, name="stage", tag="stage")
                pss = [qpool.tile([128, 400], mybir.dt.float32, name=f"ps{t}", tag=f"ps{t}")
                       for t in range(4)]
                first = [True, True]
                for ui, u in enumerate(UNITS):
                    grp = ui % 2
                    if u[0] == "z":
                        _, dx, dy, zg = u
                        kind, dyy, zo = "z", dy, 2 * zg
                    else:
                        _, dx, yg, _ = u
                        kind, dyy, zo = "y", 2 * yg, 4
                    pl3 = plane(kind, px + dx)
                    for ty in range(4):
                        yb = ty * 10 + dyy
                        rhs = pl3[:, yb:yb + 10, zo:zo + 40]
                        nc.tensor.matmul(
                            pss[ty][grp * 64:(grp + 1) * 64, :],
                            wt_sb[:, ui * DIM:(ui + 1) * DIM],
                            rhs,
                            start=first[grp],
                            stop=(ui >= NU - 2),
                            tile_position=(0, grp * 64),
                        )
                    first[grp] = False
                for ty in range(4):
                    nc.vector.tensor_copy(stage[:, ty * 400:(ty + 1) * 400], pss[ty][:])
                    nc.scalar.dma_start(
                        y_d.ap()[:, px * 1600 + ty * 400:px * 1600 + (ty + 1) * 400],
                        stage[:, ty * 400:(ty + 1) * 400])

    orig = nc.to_json_bytes
    nc.to_json_bytes = functools.wraps(orig)(lambda: _split_sync_waits_json(orig()))
    return nc


def kernel(x, linear_weight, weight, _trace=False):
    x = np.asarray(x, np.float32)
    k = _build_tp_kernel(np.asarray(linear_weight), np.asarray(weight))
    wt = _pack_weights(k)

    in_maps = []
    for core in range(8):
        b, cx = divmod(core, 4)
        in_maps.append({"xs": _build_slab(x[b], cx), "wt": wt})

    nc = _build_program()
    res = run_bass_kernel_spmd(nc, in_maps, core_ids=list(range(8)), trace=_trace)

    y = np.empty((2, 40, 40, 40, DIM), np.float32)
    for core in range(8):
        b, cx = divmod(core, 4)
        yc = res.results[core]["y"]
        s = (yc[:64] + yc[64:]).reshape(DIM, 10, 4, 10, 40)
        y[b, cx * 10:(cx + 1) * 10] = s.transpose(1, 2, 3, 4, 0).reshape(10, 40, 40, DIM)
    if _trace:
        kernel.last_results = res
    return y


# revision 8
# speedup vs baseline: 1.0715x; 1.0715x over previous
"""Trainium2 Bass kernel for nn_Convolution_77111842832763.

3D conv 5x5x5 SAME, 64->64 channels, input [2,40,40,40,64] fp32, plus an
irrep-wise linear self-connection (folded into the conv's center tap).

Strategy (8 NeuronCores, data-parallel):
  - Shard: core = batch(2) x x-chunk(4); each core computes a [10,40,40,64]
    output slab from a zero-padded [14,44,44,64] input slab (halo 2).
  - Host builds the 5^3 x 64 x 64 tensor-product kernel exactly (float64),
    folds the self-connection into the center tap, and packs per-tap weight
    blocks; weights/slabs are cast to fp16 on host (device matmuls run fp16
    with fp32 PSUM accumulation; measured end-to-end rel-err ~2.9e-4).
  - Device: TWO channel-major slab copies in SBUF [128, 14*44*44]:
      slab_z: partitions 0-63 = slab, 64-127 = slab shifted +1 z-voxel
      slab_y: partitions 0-63 = slab, 64-127 = slab shifted +1 y-row
    K=128 packing: the 125 taps become 65 matmul units per output tile:
      50 z-pair units  (dx,dy, dz in {(0,1),(2,3)})        -> slab_z
      15 y-pair units  (dx, dz=4 slice, dy in {(0,1),(2,3),(4,zero)}) -> slab_y
  - Units alternate between PE column groups 0-63/64-127 (2x column
    tiling) accumulating into psum[0:64]/psum[64:128]; the two partial
    sums are DMA'd out separately and added on host.
  - Output tile = one x-plane quarter: 10 y-rows x 40 z = 400 voxels
    (moving free dim 400, one PSUM bank).  40 tiles per core.
  - DMA: input planes on the SP HWDGE ring, weights/outputs on the ACT
    HWDGE ring (parallel rings).  A JSON post-pass splits multi-wait
    instructions (this walrus build allows one sync wait per instruction).
"""

import functools
import json
import math
from contextlib import ExitStack

import numpy as np

import concourse.bass as bass
import concourse.mybir as mybir
import concourse.tile as tile
from concourse.bass_utils import run_bass_kernel_spmd

MUL = 16
DIM = 64
NB = 8
PX, PY, PZ = 14, 44, 44          # padded slab dims
PLANE = PY * PZ                   # 1936
SLAB = PX * PLANE                 # 27104
SLAB_PAD = SLAB + 48              # tail zeros so the +1z and +44y views stay in-bounds
# unit list: ("z", dx, dy, zg) -> taps (dx,dy,2*zg)+(dx,dy,2*zg+1) via slab_z
#            ("y", dx, yg)     -> taps (dx,2*yg,4)+(dx,2*yg+1,4)   via slab_y
UNITS = [("z", dx, dy, zg) for dx in range(5) for dy in range(5) for zg in range(2)]
UNITS += [("y", dx, yg, 0) for dx in range(5) for yg in range(3)]
NU = len(UNITS)                   # 65


def _split_sync_waits_json(raw: bytes) -> bytes:
    """Hoist all but the last sync wait of each instruction onto preceding
    same-engine EventSemaphore instructions (engines execute in order, so
    this is semantically identical)."""
    m = json.loads(raw)
    ctr = 0
    for fn in m.get("functions", []):
        for blk in fn.get("blocks", []):
            out = []
            for inst in blk.get("instructions", []):
                si = inst.get("sync_info")
                ow = (si or {}).get("on_wait") or []
                if len(ow) > 1:
                    for w in ow[:-1]:
                        ctr += 1
                        out.append({
                            "debug": inst.get("debug", 0),
                            "engine": inst["engine"],
                            "ins": [],
                            "outs": [],
                            "name": f"SWX-{ctr}",
                            "opcode": "EventSemaphore",
                            "sync_info": {"on_update": [], "on_wait": [w]},
                        })
                    si["on_wait"] = [ow[-1]]
                out.append(inst)
            blk["instructions"] = out
    return json.dumps(m).encode()


def _build_tp_kernel(linear_weight: np.ndarray, weight: np.ndarray) -> np.ndarray:
    """Mirror reference.py's CG kernel construction in float64.
    Returns k[5,5,5,64,64] ([dx,dy,dz,in,out]) with the self-connection
    folded into the center tap."""
    lw = linear_weight.astype(np.float64)
    w8 = weight.astype(np.float64)
    ax = np.arange(-2.0, 3.0)
    gx, gy, gz = np.meshgrid(ax, ax, ax, indexing="ij")
    lattice = np.stack([gx, gy, gz], axis=-1)            # [5,5,5,3]
    rad = np.linalg.norm(lattice, axis=-1)
    values = np.linspace(0.0, 2.5, NB + 2)[1:-1]
    step = 2.5 / (NB + 1)
    diff = (rad[..., None] - values) / step
    den = np.maximum(1.0 - diff * diff, 1e-9)
    emb = np.where(np.abs(diff) < 1.0, 1.14136 * np.exp(2.0 - 1.0 / den), 0.0)
    n = rad[..., None]
    unit = np.where(n > 0, lattice / np.where(n > 0, n, 1.0), 0.0)
    sh = np.concatenate([np.ones((5, 5, 5, 1)), math.sqrt(3.0) * unit], -1)
    L = 125
    w = (emb.reshape(L, NB) @ w8) / float(L)             # [125, 1024]
    W = w.reshape(L, 4, MUL, MUL)
    shf = sh.reshape(L, 4)
    y0, y1 = shf[:, 0], shf[:, 1:4]
    a = 1.0 / math.sqrt(2.0 * MUL)
    eye3 = np.eye(3)
    Rss = a * W[:, 0] * y0[:, None, None]
    Rsv = a * np.einsum("luw,lm->luwm", W[:, 1], y1).reshape(L, MUL, 3 * MUL)
    Rvv = a * np.einsum("luw,l,mn->lumwn", W[:, 2], y0, eye3).reshape(L, 3 * MUL, 3 * MUL)
    Rvs = (a / math.sqrt(3.0)) * np.einsum("luw,lm->lumw", W[:, 3], y1).reshape(L, 3 * MUL, MUL)
    k = np.concatenate(
        [np.concatenate([Rss, Rsv], -1), np.concatenate([Rvs, Rvv], -1)], 1
    ).reshape(5, 5, 5, DIM, DIM)
    # self-connection: irrep-wise linear, folded into center tap
    Wl = lw.reshape(2, MUL, MUL) / math.sqrt(MUL)
    sc = np.zeros((DIM, DIM))
    sc[:MUL, :MUL] = Wl[0]
    for m in range(3):
        idx = MUL + np.arange(MUL) * 3 + m
        sc[np.ix_(idx, idx)] = Wl[1]
    k = k.copy()
    k[2, 2, 2] += sc
    return k


def _pack_weights(k: np.ndarray) -> np.ndarray:
    """[128, 65*64] fp16 per-unit weight blocks (rows 64-127 = paired tap,
    zeros when unpaired)."""
    Wp = np.zeros((128, NU * DIM), np.float64)
    for ui, u in enumerate(UNITS):
        s = ui * DIM
        if u[0] == "z":
            _, dx, dy, zg = u
            Wp[0:64, s:s + DIM] = k[dx, dy, 2 * zg]
            Wp[64:128, s:s + DIM] = k[dx, dy, 2 * zg + 1]
        else:
            _, dx, yg, _ = u
            Wp[0:64, s:s + DIM] = k[dx, 2 * yg, 4]
            if 2 * yg + 1 < 5:
                Wp[64:128, s:s + DIM] = k[dx, 2 * yg + 1, 4]
    return Wp.astype(np.float16)


def _build_slabs(xb: np.ndarray, cx: int):
    """Channel-major zero-padded fp16 slab copies for x-chunk cx of
    batch-slice xb [40,40,40,64]:
      xz [128, SLAB_PAD]: rows 0-63 slab, rows 64-127 slab shifted +1 z-voxel
      xy [128, SLAB_PAD]: rows 0-63 slab, rows 64-127 slab shifted +1 y-row
    (full-128-partition DMA runs at twice the rate of a 64-partition one)."""
    pad = np.zeros((PX, PY, PZ, DIM), np.float32)
    x0 = cx * 10 - 2
    lo, hi = max(0, x0), min(40, x0 + PX)
    pad[lo - x0:hi - x0, 2:42, 2:42, :] = xb[lo:hi]
    xs = np.zeros((DIM, SLAB_PAD), np.float16)
    xs[:, :SLAB] = np.ascontiguousarray(
        pad.transpose(3, 0, 1, 2)).reshape(DIM, SLAB).astype(np.float16)
    xz = np.zeros((128, SLAB_PAD), np.float16)
    xy = np.zeros((128, SLAB_PAD), np.float16)
    xz[:DIM] = xs
    xy[:DIM] = xs
    xz[DIM:, :SLAB_PAD - 1] = xs[:, 1:]
    xy[DIM:, :SLAB_PAD - PY] = xs[:, PY:]
    return xz, xy


def _build_program():
    nc = bass.Bass("TRN2", target_bir_lowering=False, debug=False)
    xz_d = nc.dram_tensor("xz", [128, SLAB_PAD], mybir.dt.float16, kind="ExternalInput")
    xy_d = nc.dram_tensor("xy", [128, SLAB_PAD], mybir.dt.float16, kind="ExternalInput")
    wt_d = nc.dram_tensor("wt", [128, NU * DIM], mybir.dt.float16, kind="ExternalInput")
    y_d = nc.dram_tensor("y", [128, 16000], mybir.dt.float32, kind="ExternalOutput")

    with tile.TileContext(nc) as tc:
        with ExitStack() as ctx:
            wpool = ctx.enter_context(tc.tile_pool(name="wts", bufs=1))
            ppool = ctx.enter_context(tc.tile_pool(name="planes", bufs=1))
            spool = ctx.enter_context(tc.tile_pool(name="stage", bufs=4))
            qpool = ctx.enter_context(tc.tile_pool(name="psum", bufs=2, space="PSUM"))

            wt_sb = wpool.tile([128, NU * DIM], mybir.dt.float16)
            nc.scalar.dma_start(wt_sb[:], wt_d.ap())

            # plane groups: 0-4 (gates the first output plane) and 5-13,
            # one batched DMA per slab-copy half per group
            G0, G1 = 5, PX - 5
            gz0 = ppool.tile([128, G0 * PLANE], mybir.dt.float16, name="gz0", tag="gz0")
            gz1 = ppool.tile([128, G1 * PLANE], mybir.dt.float16, name="gz1", tag="gz1")
            gy0 = ppool.tile([128, G0 * PLANE], mybir.dt.float16, name="gy0", tag="gy0")
            gy1 = ppool.tile([128, G1 * PLANE], mybir.dt.float16, name="gy1", tag="gy1")
            s0, s1 = G0 * PLANE, SLAB
            nc.sync.dma_start(gz0[:], xz_d.ap()[:, 0:s0])
            nc.sync.dma_start(gz1[:], xz_d.ap()[:, s0:s1])
            nc.scalar.dma_start(gy0[:], xy_d.ap()[:, 0:s0])
            nc.scalar.dma_start(gy1[:], xy_d.ap()[:, s0:s1])

            def plane(kind, i):
                g = (gz0 if i < G0 else gz1) if kind == "z" else (gy0 if i < G0 else gy1)
                off = i if i < G0 else i - G0
                return g[:].rearrange("p (x y z) -> p x y z", y=PY, z=PZ)[:, off]

            for px in range(10):
                stage = spool.tile([128, 1600], mybir.dt.float32, name="stage", tag="stage")
                pss = [qpool.tile([128, 400], mybir.dt.float32, name=f"ps{t}", tag=f"ps{t}")
                       for t in range(4)]
                first = [True, True]
                for ui, u in enumerate(UNITS):
                    grp = ui % 2
                    if u[0] == "z":
                        _, dx, dy, zg = u
                        kind, dyy, zo = "z", dy, 2 * zg
                    else:
                        _, dx, yg, _ = u
                        kind, dyy, zo = "y", 2 * yg, 4
                    pl3 = plane(kind, px + dx)
                    for ty in range(4):
                        yb = ty * 10 + dyy
                        rhs = pl3[:, yb:yb + 10, zo:zo + 40]
                        nc.tensor.matmul(
                            pss[ty][grp * 64:(grp + 1) * 64, :],
                            wt_sb[:, ui * DIM:(ui + 1) * DIM],
                            rhs,
                            start=first[grp],
                            stop=(ui >= NU - 2),
                            tile_position=(0, grp * 64),
                        )
                    first[grp] = False
                for ty in range(4):
                    nc.vector.tensor_copy(stage[:, ty * 400:(ty + 1) * 400], pss[ty][:])
                    nc.scalar.dma_start(
                        y_d.ap()[:, px * 1600 + ty * 400:px * 1600 + (ty + 1) * 400],
                        stage[:, ty * 400:(ty + 1) * 400])

    orig = nc.to_json_bytes
    nc.to_json_bytes = functools.wraps(orig)(lambda: _split_sync_waits_json(orig()))
    return nc


def kernel(x, linear_weight, weight, _trace=False):
    x = np.asarray(x, np.float32)
    k = _build_tp_kernel(np.asarray(linear_weight), np.asarray(weight))
    wt = _pack_weights(k)

    in_maps = []
    for core in range(8):
        b, cx = divmod(core, 4)
        xz, xy = _build_slabs(x[b], cx)
        in_maps.append({"xz": xz, "xy": xy, "wt": wt})

    nc = _build_program()
    res = run_bass_kernel_spmd(nc, in_maps, core_ids=list(range(8)), trace=_trace)

    y = np.empty((2, 40, 40, 40, DIM), np.float32)
    for core in range(8):
        b, cx = divmod(core, 4)
        yc = res.results[core]["y"]
        s = (yc[:64] + yc[64:]).reshape(DIM, 10, 4, 10, 40)
        y[b, cx * 10:(cx + 1) * 10] = s.transpose(1, 2, 3, 4, 0).reshape(10, 40, 40, DIM)
    if _trace:
        kernel.last_results = res
    return y


# revision 9
# speedup vs baseline: 1.1276x; 1.0524x over previous
"""Trainium2 Bass kernel for nn_Convolution_77111842832763.

3D conv 5x5x5 SAME, 64->64 channels, input [2,40,40,40,64] fp32, plus an
irrep-wise linear self-connection (folded into the conv's center tap).

Strategy (8 NeuronCores, data-parallel):
  - Shard: core = batch(2) x x-chunk(4); each core computes a [10,40,40,64]
    output slab from a zero-padded [14,44,44,64] input slab (halo 2).
  - Host builds the 5^3 x 64 x 64 tensor-product kernel exactly (float64),
    folds the self-connection into the center tap, and packs per-tap weight
    blocks; weights/slabs are cast to fp16 on host (device matmuls run fp16
    with fp32 PSUM accumulation; measured end-to-end rel-err ~2.9e-4).
  - Device: TWO channel-major slab copies in SBUF [128, 14*44*44]:
      slab_z: partitions 0-63 = slab, 64-127 = slab shifted +1 z-voxel
      slab_y: partitions 0-63 = slab, 64-127 = slab shifted +1 y-row
    K=128 packing: the 125 taps become 65 matmul units per output tile:
      50 z-pair units  (dx,dy, dz in {(0,1),(2,3)})        -> slab_z
      15 y-pair units  (dx, dz=4 slice, dy in {(0,1),(2,3),(4,zero)}) -> slab_y
  - Units alternate between PE column groups 0-63/64-127 (2x column
    tiling) accumulating into psum[0:64]/psum[64:128]; the two partial
    sums are DMA'd out separately and added on host.
  - Output tile = one x-plane quarter: 10 y-rows x 40 z = 400 voxels
    (moving free dim 400, one PSUM bank).  40 tiles per core.
  - DMA: input planes on the SP HWDGE ring, weights/outputs on the ACT
    HWDGE ring (parallel rings).  A JSON post-pass splits multi-wait
    instructions (this walrus build allows one sync wait per instruction).
"""

import functools
import json
import math
from contextlib import ExitStack

import numpy as np

import concourse.bass as bass
import concourse.mybir as mybir
import concourse.tile as tile
from concourse.bass_utils import run_bass_kernel_spmd

MUL = 16
DIM = 64
NB = 8
PX, PY, PZ = 14, 44, 44          # padded slab dims
PLANE = PY * PZ                   # 1936
SLAB = PX * PLANE                 # 27104
SLAB_PAD = SLAB + 48              # tail zeros so the +1z and +44y views stay in-bounds
# unit list: ("z", dx, dy, zg) -> taps (dx,dy,2*zg)+(dx,dy,2*zg+1) via slab_z
#            ("y", dx, yg)     -> taps (dx,2*yg,4)+(dx,2*yg+1,4)   via slab_y
UNITS = [("z", dx, dy, zg) for dx in range(5) for dy in range(5) for zg in range(2)]
UNITS += [("y", dx, yg, 0) for dx in range(5) for yg in range(3)]
NU = len(UNITS)                   # 65


def _split_sync_waits_json(raw: bytes) -> bytes:
    """Hoist all but the last sync wait of each instruction onto preceding
    same-engine EventSemaphore instructions (engines execute in order, so
    this is semantically identical)."""
    m = json.loads(raw)
    ctr = 0
    for fn in m.get("functions", []):
        for blk in fn.get("blocks", []):
            out = []
            for inst in blk.get("instructions", []):
                si = inst.get("sync_info")
                ow = (si or {}).get("on_wait") or []
                if len(ow) > 1:
                    for w in ow[:-1]:
                        ctr += 1
                        out.append({
                            "debug": inst.get("debug", 0),
                            "engine": inst["engine"],
                            "ins": [],
                            "outs": [],
                            "name": f"SWX-{ctr}",
                            "opcode": "EventSemaphore",
                            "sync_info": {"on_update": [], "on_wait": [w]},
                        })
                    si["on_wait"] = [ow[-1]]
                out.append(inst)
            blk["instructions"] = out
    return json.dumps(m).encode()


def _build_tp_kernel(linear_weight: np.ndarray, weight: np.ndarray) -> np.ndarray:
    """Mirror reference.py's CG kernel construction in float64.
    Returns k[5,5,5,64,64] ([dx,dy,dz,in,out]) with the self-connection
    folded into the center tap."""
    lw = linear_weight.astype(np.float64)
    w8 = weight.astype(np.float64)
    ax = np.arange(-2.0, 3.0)
    gx, gy, gz = np.meshgrid(ax, ax, ax, indexing="ij")
    lattice = np.stack([gx, gy, gz], axis=-1)            # [5,5,5,3]
    rad = np.linalg.norm(lattice, axis=-1)
    values = np.linspace(0.0, 2.5, NB + 2)[1:-1]
    step = 2.5 / (NB + 1)
    diff = (rad[..., None] - values) / step
    den = np.maximum(1.0 - diff * diff, 1e-9)
    emb = np.where(np.abs(diff) < 1.0, 1.14136 * np.exp(2.0 - 1.0 / den), 0.0)
    n = rad[..., None]
    unit = np.where(n > 0, lattice / np.where(n > 0, n, 1.0), 0.0)
    sh = np.concatenate([np.ones((5, 5, 5, 1)), math.sqrt(3.0) * unit], -1)
    L = 125
    w = (emb.reshape(L, NB) @ w8) / float(L)             # [125, 1024]
    W = w.reshape(L, 4, MUL, MUL)
    shf = sh.reshape(L, 4)
    y0, y1 = shf[:, 0], shf[:, 1:4]
    a = 1.0 / math.sqrt(2.0 * MUL)
    eye3 = np.eye(3)
    Rss = a * W[:, 0] * y0[:, None, None]
    Rsv = a * np.einsum("luw,lm->luwm", W[:, 1], y1).reshape(L, MUL, 3 * MUL)
    Rvv = a * np.einsum("luw,l,mn->lumwn", W[:, 2], y0, eye3).reshape(L, 3 * MUL, 3 * MUL)
    Rvs = (a / math.sqrt(3.0)) * np.einsum("luw,lm->lumw", W[:, 3], y1).reshape(L, 3 * MUL, MUL)
    k = np.concatenate(
        [np.concatenate([Rss, Rsv], -1), np.concatenate([Rvs, Rvv], -1)], 1
    ).reshape(5, 5, 5, DIM, DIM)
    # self-connection: irrep-wise linear, folded into center tap
    Wl = lw.reshape(2, MUL, MUL) / math.sqrt(MUL)
    sc = np.zeros((DIM, DIM))
    sc[:MUL, :MUL] = Wl[0]
    for m in range(3):
        idx = MUL + np.arange(MUL) * 3 + m
        sc[np.ix_(idx, idx)] = Wl[1]
    k = k.copy()
    k[2, 2, 2] += sc
    return k


def _pack_weights(k: np.ndarray) -> np.ndarray:
    """[128, 65*64] fp16 per-unit weight blocks (rows 64-127 = paired tap,
    zeros when unpaired)."""
    Wp = np.zeros((128, NU * DIM), np.float64)
    for ui, u in enumerate(UNITS):
        s = ui * DIM
        if u[0] == "z":
            _, dx, dy, zg = u
            Wp[0:64, s:s + DIM] = k[dx, dy, 2 * zg]
            Wp[64:128, s:s + DIM] = k[dx, dy, 2 * zg + 1]
        else:
            _, dx, yg, _ = u
            Wp[0:64, s:s + DIM] = k[dx, 2 * yg, 4]
            if 2 * yg + 1 < 5:
                Wp[64:128, s:s + DIM] = k[dx, 2 * yg + 1, 4]
    return Wp.astype(np.float16)


def _build_slabs(xb: np.ndarray, cx: int):
    """Channel-major zero-padded fp16 slab copies for x-chunk cx of
    batch-slice xb [40,40,40,64]:
      xz [128, SLAB_PAD]: rows 0-63 slab, rows 64-127 slab shifted +1 z-voxel
      xy [128, SLAB_PAD]: rows 0-63 slab, rows 64-127 slab shifted +1 y-row
    (full-128-partition DMA runs at twice the rate of a 64-partition one)."""
    pad = np.zeros((PX, PY, PZ, DIM), np.float32)
    x0 = cx * 10 - 2
    lo, hi = max(0, x0), min(40, x0 + PX)
    pad[lo - x0:hi - x0, 2:42, 2:42, :] = xb[lo:hi]
    xs = np.zeros((DIM, SLAB_PAD), np.float16)
    xs[:, :SLAB] = np.ascontiguousarray(
        pad.transpose(3, 0, 1, 2)).reshape(DIM, SLAB).astype(np.float16)
    xz = np.zeros((128, SLAB_PAD), np.float16)
    xy = np.zeros((128, SLAB_PAD), np.float16)
    xz[:DIM] = xs
    xy[:DIM] = xs
    xz[DIM:, :SLAB_PAD - 1] = xs[:, 1:]
    xy[DIM:, :SLAB_PAD - PY] = xs[:, PY:]
    return xz, xy


def _build_program():
    nc = bass.Bass("TRN2", target_bir_lowering=False, debug=False)
    xz_d = nc.dram_tensor("xz", [128, SLAB_PAD], mybir.dt.float16, kind="ExternalInput")
    xy_d = nc.dram_tensor("xy", [128, SLAB_PAD], mybir.dt.float16, kind="ExternalInput")
    wt_d = nc.dram_tensor("wt", [128, NU * DIM], mybir.dt.float16, kind="ExternalInput")
    y_d = nc.dram_tensor("y", [128, 16000], mybir.dt.float32, kind="ExternalOutput")

    with tile.TileContext(nc) as tc:
        with ExitStack() as ctx:
            wpool = ctx.enter_context(tc.tile_pool(name="wts", bufs=1))
            ppool = ctx.enter_context(tc.tile_pool(name="planes", bufs=1))
            spool = ctx.enter_context(tc.tile_pool(name="stage", bufs=4))
            qpool = ctx.enter_context(tc.tile_pool(name="psum", bufs=2, space="PSUM"))

            # weights in 4 chunks so the first matmuls gate on 1/4 of it
            wt_sb = wpool.tile([128, NU * DIM], mybir.dt.float16)
            WC = NU * DIM // 4
            for c in range(4):
                nc.scalar.dma_start(wt_sb[:, c * WC:(c + 1) * WC],
                                    wt_d.ap()[:, c * WC:(c + 1) * WC])

            # per-plane full-partition DMAs: z-copy on the SP ring, y-copy on
            # the ACT ring, so the PE starts after wt chunk 0 + z-plane 0
            zplanes, yplanes = [], []
            for i in range(PX):
                o = i * PLANE
                tz = ppool.tile([128, PLANE], mybir.dt.float16, name=f"pz{i}", tag=f"pz{i}")
                nc.sync.dma_start(tz[:], xz_d.ap()[:, o:o + PLANE])
                zplanes.append(tz)
                ty_ = ppool.tile([128, PLANE], mybir.dt.float16, name=f"py{i}", tag=f"py{i}")
                nc.scalar.dma_start(ty_[:], xy_d.ap()[:, o:o + PLANE])
                yplanes.append(ty_)

            def plane(kind, i):
                g = zplanes[i] if kind == "z" else yplanes[i]
                return g[:].rearrange("p (y z) -> p y z", y=PY)

            def unit_src(u, px, ty):
                if u[0] == "z":
                    _, dx, dy, zg = u
                    kind, dyy, zo = "z", dy, 2 * zg
                else:
                    _, dx, yg, _ = u
                    kind, dyy, zo = "y", 2 * yg, 4
                yb = ty * 10 + dyy
                return plane(kind, px + dx)[:, yb:yb + 10, zo:zo + 40]

            for px in range(10):
                stage = spool.tile([128, 1600], mybir.dt.float32, name="stage", tag="stage")
                last = px == 9
                if not last:
                    # ty-innermost: 4 matmuls share one weight load
                    pss = [qpool.tile([128, 400], mybir.dt.float32, name=f"ps{t}", tag=f"ps{t}")
                           for t in range(4)]
                    first = [True, True]
                    for ui, u in enumerate(UNITS):
                        grp = ui % 2
                        for ty in range(4):
                            nc.tensor.matmul(
                                pss[ty][grp * 64:(grp + 1) * 64, :],
                                wt_sb[:, ui * DIM:(ui + 1) * DIM],
                                unit_src(u, px, ty),
                                start=first[grp],
                                stop=(ui >= NU - 2),
                                tile_position=(0, grp * 64),
                            )
                        first[grp] = False
                    for ty in range(4):
                        nc.vector.tensor_copy(stage[:, ty * 400:(ty + 1) * 400], pss[ty][:])
                        nc.scalar.dma_start(
                            y_d.ap()[:, px * 1600 + ty * 400:px * 1600 + (ty + 1) * 400],
                            stage[:, ty * 400:(ty + 1) * 400])
                else:
                    # last plane: per-ty accumulation so only the final
                    # quarter's evacuation is exposed at the kernel tail
                    for ty in range(4):
                        ps = qpool.tile([128, 400], mybir.dt.float32, name=f"ps{ty}", tag=f"ps{ty}")
                        first = [True, True]
                        for ui, u in enumerate(UNITS):
                            grp = ui % 2
                            nc.tensor.matmul(
                                ps[grp * 64:(grp + 1) * 64, :],
                                wt_sb[:, ui * DIM:(ui + 1) * DIM],
                                unit_src(u, px, ty),
                                start=first[grp],
                                stop=(ui >= NU - 2),
                                tile_position=(0, grp * 64),
                            )
                            first[grp] = False
                        nc.vector.tensor_copy(stage[:, ty * 400:(ty + 1) * 400], ps[:])
                        nc.scalar.dma_start(
                            y_d.ap()[:, px * 1600 + ty * 400:px * 1600 + (ty + 1) * 400],
                            stage[:, ty * 400:(ty + 1) * 400])

    orig = nc.to_json_bytes
    nc.to_json_bytes = functools.wraps(orig)(lambda: _split_sync_waits_json(orig()))
    return nc


def kernel(x, linear_weight, weight, _trace=False):
    x = np.asarray(x, np.float32)
    k = _build_tp_kernel(np.asarray(linear_weight), np.asarray(weight))
    wt = _pack_weights(k)

    in_maps = []
    for core in range(8):
        b, cx = divmod(core, 4)
        xz, xy = _build_slabs(x[b], cx)
        in_maps.append({"xz": xz, "xy": xy, "wt": wt})

    nc = _build_program()
    res = run_bass_kernel_spmd(nc, in_maps, core_ids=list(range(8)), trace=_trace)

    y = np.empty((2, 40, 40, 40, DIM), np.float32)
    for core in range(8):
        b, cx = divmod(core, 4)
        yc = res.results[core]["y"]
        s = (yc[:64] + yc[64:]).reshape(DIM, 10, 4, 10, 40)
        y[b, cx * 10:(cx + 1) * 10] = s.transpose(1, 2, 3, 4, 0).reshape(10, 40, 40, DIM)
    if _trace:
        kernel.last_results = res
    return y


# revision 15
# speedup vs baseline: 1.1285x; 1.0008x over previous
"""Trainium2 Bass kernel for nn_Convolution_77111842832763.

3D conv 5x5x5 SAME, 64->64 channels, input [2,40,40,40,64] fp32, plus an
irrep-wise linear self-connection (folded into the conv's center tap).

Strategy (8 NeuronCores, data-parallel):
  - Shard: core = batch(2) x x-chunk(4); each core computes a [10,40,40,64]
    output slab from a zero-padded [14,44,44,64] input slab (halo 2).
  - Host builds the 5^3 x 64 x 64 tensor-product kernel exactly (float64),
    folds the self-connection into the center tap, and packs per-tap weight
    blocks; weights/slabs are cast to fp16 on host (device matmuls run fp16
    with fp32 PSUM accumulation; measured end-to-end rel-err ~2.9e-4).
  - Device: TWO channel-major slab copies in SBUF [128, 14*44*44]:
      slab_z: partitions 0-63 = slab, 64-127 = slab shifted +1 z-voxel
      slab_y: partitions 0-63 = slab, 64-127 = slab shifted +1 y-row
    K=128 packing: the 125 taps become 65 matmul units per output tile:
      50 z-pair units  (dx,dy, dz in {(0,1),(2,3)})        -> slab_z
      15 y-pair units  (dx, dz=4 slice, dy in {(0,1),(2,3),(4,zero)}) -> slab_y
  - Units alternate between PE column groups 0-63/64-127 (2x column
    tiling) accumulating into psum[0:64]/psum[64:128]; the two partial
    sums are DMA'd out separately and added on host.
  - Output tile = one x-plane quarter: 10 y-rows x 40 z = 400 voxels
    (moving free dim 400, one PSUM bank).  40 tiles per core.
  - DMA: input planes on the SP HWDGE ring, weights/outputs on the ACT
    HWDGE ring (parallel rings).  A JSON post-pass splits multi-wait
    instructions (this walrus build allows one sync wait per instruction).
"""

import functools
import json
import math
from contextlib import ExitStack

import numpy as np

import concourse.bass as bass
import concourse.mybir as mybir
import concourse.tile as tile
from concourse.bass_utils import run_bass_kernel_spmd

MUL = 16
DIM = 64
NB = 8
PX, PY, PZ = 14, 44, 44          # padded slab dims
PLANE = PY * PZ                   # 1936
SLAB = PX * PLANE                 # 27104
SLAB_PAD = SLAB + 48              # tail zeros so the +1z and +44y views stay in-bounds
# unit list: ("z", dx, dy, zg) -> taps (dx,dy,2*zg)+(dx,dy,2*zg+1) via slab_z
#            ("y", dx, yg)     -> taps (dx,2*yg,4)+(dx,2*yg+1,4)   via slab_y
#            ("x", xg)         -> taps (2*xg,4,4)+(2*xg+1,4,4)     via slab_x
UNITS = [("z", dx, dy, zg) for dx in range(5) for dy in range(5) for zg in range(2)]
UNITS += [("y", dx, yg, 0) for dx in range(5) for yg in range(2)]
UNITS += [("x", xg, 0, 0) for xg in range(3)]
NU = len(UNITS)                   # 63


def _split_sync_waits_json(raw: bytes) -> bytes:
    """Hoist all but the last sync wait of each instruction onto preceding
    same-engine EventSemaphore instructions (engines execute in order, so
    this is semantically identical)."""
    m = json.loads(raw)
    ctr = 0
    for fn in m.get("functions", []):
        for blk in fn.get("blocks", []):
            out = []
            for inst in blk.get("instructions", []):
                si = inst.get("sync_info")
                ow = (si or {}).get("on_wait") or []
                if len(ow) > 1:
                    for w in ow[:-1]:
                        ctr += 1
                        out.append({
                            "debug": inst.get("debug", 0),
                            "engine": inst["engine"],
                            "ins": [],
                            "outs": [],
                            "name": f"SWX-{ctr}",
                            "opcode": "EventSemaphore",
                            "sync_info": {"on_update": [], "on_wait": [w]},
                        })
                    si["on_wait"] = [ow[-1]]
                out.append(inst)
            blk["instructions"] = out
    return json.dumps(m).encode()


def _build_tp_kernel(linear_weight: np.ndarray, weight: np.ndarray) -> np.ndarray:
    """Mirror reference.py's CG kernel construction in float64.
    Returns k[5,5,5,64,64] ([dx,dy,dz,in,out]) with the self-connection
    folded into the center tap."""
    lw = linear_weight.astype(np.float64)
    w8 = weight.astype(np.float64)
    ax = np.arange(-2.0, 3.0)
    gx, gy, gz = np.meshgrid(ax, ax, ax, indexing="ij")
    lattice = np.stack([gx, gy, gz], axis=-1)            # [5,5,5,3]
    rad = np.linalg.norm(lattice, axis=-1)
    values = np.linspace(0.0, 2.5, NB + 2)[1:-1]
    step = 2.5 / (NB + 1)
    diff = (rad[..., None] - values) / step
    den = np.maximum(1.0 - diff * diff, 1e-9)
    emb = np.where(np.abs(diff) < 1.0, 1.14136 * np.exp(2.0 - 1.0 / den), 0.0)
    n = rad[..., None]
    unit = np.where(n > 0, lattice / np.where(n > 0, n, 1.0), 0.0)
    sh = np.concatenate([np.ones((5, 5, 5, 1)), math.sqrt(3.0) * unit], -1)
    L = 125
    w = (emb.reshape(L, NB) @ w8) / float(L)             # [125, 1024]
    W = w.reshape(L, 4, MUL, MUL)
    shf = sh.reshape(L, 4)
    y0, y1 = shf[:, 0], shf[:, 1:4]
    a = 1.0 / math.sqrt(2.0 * MUL)
    eye3 = np.eye(3)
    Rss = a * W[:, 0] * y0[:, None, None]
    Rsv = a * np.einsum("luw,lm->luwm", W[:, 1], y1).reshape(L, MUL, 3 * MUL)
    Rvv = a * np.einsum("luw,l,mn->lumwn", W[:, 2], y0, eye3).reshape(L, 3 * MUL, 3 * MUL)
    Rvs = (a / math.sqrt(3.0)) * np.einsum("luw,lm->lumw", W[:, 3], y1).reshape(L, 3 * MUL, MUL)
    k = np.concatenate(
        [np.concatenate([Rss, Rsv], -1), np.concatenate([Rvs, Rvv], -1)], 1
    ).reshape(5, 5, 5, DIM, DIM)
    # self-connection: irrep-wise linear, folded into center tap
    Wl = lw.reshape(2, MUL, MUL) / math.sqrt(MUL)
    sc = np.zeros((DIM, DIM))
    sc[:MUL, :MUL] = Wl[0]
    for m in range(3):
        idx = MUL + np.arange(MUL) * 3 + m
        sc[np.ix_(idx, idx)] = Wl[1]
    k = k.copy()
    k[2, 2, 2] += sc
    return k


def _pack_weights(k: np.ndarray) -> np.ndarray:
    """[128, 65*64] fp16 per-unit weight blocks (rows 64-127 = paired tap,
    zeros when unpaired)."""
    Wp = np.zeros((128, NU * DIM), np.float64)
    for ui, u in enumerate(UNITS):
        s = ui * DIM
        if u[0] == "z":
            _, dx, dy, zg = u
            Wp[0:64, s:s + DIM] = k[dx, dy, 2 * zg]
            Wp[64:128, s:s + DIM] = k[dx, dy, 2 * zg + 1]
        elif u[0] == "y":
            _, dx, yg, _ = u
            Wp[0:64, s:s + DIM] = k[dx, 2 * yg, 4]
            Wp[64:128, s:s + DIM] = k[dx, 2 * yg + 1, 4]
        else:
            _, xg, _, _ = u
            Wp[0:64, s:s + DIM] = k[2 * xg, 4, 4]
            if 2 * xg + 1 < 5:
                Wp[64:128, s:s + DIM] = k[2 * xg + 1, 4, 4]
    return Wp.astype(np.float16)


def _build_slabs(xb: np.ndarray, cx: int):
    """Channel-major zero-padded fp16 slab copies for x-chunk cx of
    batch-slice xb [40,40,40,64]:
      xz [128, SLAB_PAD]: rows 0-63 slab, rows 64-127 slab shifted +1 z-voxel
      xy [128, SLAB_PAD]: rows 0-63 slab, rows 64-127 slab shifted +1 y-row
      xx [128, SLAB_PAD]: rows 0-63 slab, rows 64-127 slab shifted +1 x-plane
    (full-128-partition DMA runs at twice the rate of a 64-partition one)."""
    pad = np.zeros((PX, PY, PZ, DIM), np.float32)
    x0 = cx * 10 - 2
    lo, hi = max(0, x0), min(40, x0 + PX)
    pad[lo - x0:hi - x0, 2:42, 2:42, :] = xb[lo:hi]
    xs = np.zeros((DIM, SLAB_PAD), np.float16)
    xs[:, :SLAB] = np.ascontiguousarray(
        pad.transpose(3, 0, 1, 2)).reshape(DIM, SLAB).astype(np.float16)
    xz = np.zeros((128, SLAB_PAD), np.float16)
    xy = np.zeros((128, SLAB_PAD), np.float16)
    xx = np.zeros((128, SLAB_PAD), np.float16)
    for a in (xz, xy, xx):
        a[:DIM] = xs
    xz[DIM:, :SLAB_PAD - 1] = xs[:, 1:]
    xy[DIM:, :SLAB_PAD - PY] = xs[:, PY:]
    xx[DIM:, :SLAB_PAD - PLANE] = xs[:, PLANE:]
    return xz, xy, xx


def _build_program():
    nc = bass.Bass("TRN2", target_bir_lowering=False, debug=False)
    xz_d = nc.dram_tensor("xz", [128, SLAB_PAD], mybir.dt.float16, kind="ExternalInput")
    xy_d = nc.dram_tensor("xy", [128, SLAB_PAD], mybir.dt.float16, kind="ExternalInput")
    xx_d = nc.dram_tensor("xx", [128, SLAB_PAD], mybir.dt.float16, kind="ExternalInput")
    wt_d = nc.dram_tensor("wt", [128, NU * DIM], mybir.dt.float16, kind="ExternalInput")
    y_d = nc.dram_tensor("y", [128, 16000], mybir.dt.float32, kind="ExternalOutput")

    with tile.TileContext(nc) as tc:
        with ExitStack() as ctx:
            wpool = ctx.enter_context(tc.tile_pool(name="wts", bufs=1))
            ppool = ctx.enter_context(tc.tile_pool(name="planes", bufs=1))
            spool = ctx.enter_context(tc.tile_pool(name="stage", bufs=4))
            qpool = ctx.enter_context(tc.tile_pool(name="psum", bufs=2, space="PSUM"))

            # weights in 4 chunks so the first matmuls gate on 1/4 of it
            wt_sb = wpool.tile([128, NU * DIM], mybir.dt.float16)
            WC = NU * DIM // 4
            for c in range(4):
                nc.scalar.dma_start(wt_sb[:, c * WC:(c + 1) * WC],
                                    wt_d.ap()[:, c * WC:(c + 1) * WC])

            # per-plane full-partition DMAs: z-copy on the SP ring, y-copy on
            # the ACT ring, so the PE starts after wt chunk 0 + z-plane 0
            zplanes, yplanes, xplanes = [], [], []
            for i in range(PX):
                o = i * PLANE
                tz = ppool.tile([128, PLANE], mybir.dt.float16, name=f"pz{i}", tag=f"pz{i}")
                nc.sync.dma_start(tz[:], xz_d.ap()[:, o:o + PLANE])
                zplanes.append(tz)
                ty_ = ppool.tile([128, PLANE], mybir.dt.float16, name=f"py{i}", tag=f"py{i}")
                nc.scalar.dma_start(ty_[:], xy_d.ap()[:, o:o + PLANE])
                yplanes.append(ty_)
                tx_ = ppool.tile([128, PLANE], mybir.dt.float16, name=f"px{i}", tag=f"px{i}")
                nc.sync.dma_start(tx_[:], xx_d.ap()[:, o:o + PLANE])
                xplanes.append(tx_)

            def plane(kind, i):
                g = {"z": zplanes, "y": yplanes, "x": xplanes}[kind][i]
                return g[:].rearrange("p (y z) -> p y z", y=PY)

            def unit_src(u, px, ty):
                if u[0] == "z":
                    _, dx, dy, zg = u
                    kind, dyy, zo = "z", dy, 2 * zg
                elif u[0] == "y":
                    _, dx, yg, _ = u
                    kind, dyy, zo = "y", 2 * yg, 4
                else:
                    _, xg, _, _ = u
                    dx, kind, dyy, zo = 2 * xg, "x", 4, 4
                yb = ty * 10 + dyy
                return plane(kind, px + dx)[:, yb:yb + 10, zo:zo + 40]

            for px in range(10):
                stage = spool.tile([128, 1600], mybir.dt.float32, name="stage", tag="stage")
                last = px == 9
                if not last:
                    # ty-innermost: 4 matmuls share one weight load
                    pss = [qpool.tile([128, 400], mybir.dt.float32, name=f"ps{t}", tag=f"ps{t}")
                           for t in range(4)]
                    first = [True, True]
                    for ui, u in enumerate(UNITS):
                        grp = ui % 2
                        for ty in range(4):
                            nc.tensor.matmul(
                                pss[ty][grp * 64:(grp + 1) * 64, :],
                                wt_sb[:, ui * DIM:(ui + 1) * DIM],
                                unit_src(u, px, ty),
                                start=first[grp],
                                stop=(ui >= NU - 2),
                                tile_position=(0, grp * 64),
                            )
                        first[grp] = False
                    for ty in range(4):
                        nc.vector.tensor_copy(stage[:, ty * 400:(ty + 1) * 400], pss[ty][:])
                        nc.scalar.dma_start(
                            y_d.ap()[:, px * 1600 + ty * 400:px * 1600 + (ty + 1) * 400],
                            stage[:, ty * 400:(ty + 1) * 400])
                else:
                    # last plane: per-ty accumulation so only the final
                    # quarter's evacuation is exposed at the kernel tail
                    for ty in range(4):
                        ps = qpool.tile([128, 400], mybir.dt.float32, name=f"ps{ty}", tag=f"ps{ty}")
                        first = [True, True]
                        for ui, u in enumerate(UNITS):
                            grp = ui % 2
                            nc.tensor.matmul(
                                ps[grp * 64:(grp + 1) * 64, :],
                                wt_sb[:, ui * DIM:(ui + 1) * DIM],
                                unit_src(u, px, ty),
                                start=first[grp],
                                stop=(ui >= NU - 2),
                                tile_position=(0, grp * 64),
                            )
                            first[grp] = False
                        nc.vector.tensor_copy(stage[:, ty * 400:(ty + 1) * 400], ps[:])
                        nc.scalar.dma_start(
                            y_d.ap()[:, px * 1600 + ty * 400:px * 1600 + (ty + 1) * 400],
                            stage[:, ty * 400:(ty + 1) * 400])

    orig = nc.to_json_bytes
    nc.to_json_bytes = functools.wraps(orig)(lambda: _split_sync_waits_json(orig()))
    return nc


def kernel(x, linear_weight, weight, _trace=False):
    x = np.asarray(x, np.float32)
    k = _build_tp_kernel(np.asarray(linear_weight), np.asarray(weight))
    wt = _pack_weights(k)

    in_maps = []
    for core in range(8):
        b, cx = divmod(core, 4)
        xz, xy, xx = _build_slabs(x[b], cx)
        in_maps.append({"xz": xz, "xy": xy, "xx": xx, "wt": wt})

    nc = _build_program()
    res = run_bass_kernel_spmd(nc, in_maps, core_ids=list(range(8)), trace=_trace)

    y = np.empty((2, 40, 40, 40, DIM), np.float32)
    for core in range(8):
        b, cx = divmod(core, 4)
        yc = res.results[core]["y"]
        s = (yc[:64] + yc[64:]).reshape(DIM, 10, 4, 10, 40)
        y[b, cx * 10:(cx + 1) * 10] = s.transpose(1, 2, 3, 4, 0).reshape(10, 40, 40, DIM)
    if _trace:
        kernel.last_results = res
    return y
